# revision 51
# baseline (speedup 1.0000x reference)
"""Trainium2 Bass kernel: segment mean+max pooling (AnchorHeightPart).

reference semantics (per (n, s) row, P=16 parts, k=512 elements, c=128 chans):
  pooled[c, p] = segsum(x*vm)[c,p]/max(segcount(vm)[p],1)
               + where(patchcount[p]>0, max(segmax(x)[c,p], -100), 0)

Device algorithm (per core, data-parallel over n: 4 n-batches/core):
  MAX path: counting-sort each row's 512 columns by label on-device
  (one-hot from host-replicated labels -> cumsum -> positions -> wrapped
  inverse via one merged local_scatter per block that also scatters the
  -1e30 boundary stripe),
  permute columns with 2-row-fused gpsimd ap_gathers (odd rows' inverse
  entries carry a +512 bias into the 1024-wide source window), one
  segmented max tensor_tensor_scan per row, then one block-batched
  ap_gather fetches all 8 rows' segment-end values.
  SUM path (no scan, no sort): PE-transposes fp16 feats chunks into PSUM
  (f16 out), Act-engine evacuates pairs to SBUF, then tiny N=16 matmuls
  against a host-transposed-label one-hot accumulate per-part sums in
  PSUM on the tensor engine.
  Combine: mean = sums * recip(count), plus indicator-masked max.
  Scheduling: label pipeline is split into 4 stages + scatter, software-
  pipelined 2 blocks ahead of the value rows; each block's ends-gather
  and combine are deferred 4 rows into the next block so the in-order
  Pool queue never waits on the last scan.
"""

import os
import sys
from contextlib import ExitStack

import numpy as np

_REPO = "/opt/trn_rl_repo"
if _REPO not in sys.path and os.path.isdir(_REPO):
    sys.path.insert(0, _REPO)

N, C, S, K = 32, 128, 30, 512
P = 16
N_CORES = 8
N_PER_CORE = N // N_CORES          # 4
ROWS = N_PER_CORE * S              # 120 rows per core
BLK = 8                            # rows per label-block
NBLK = ROWS // BLK                 # 15
SH = S // 3                        # s-rows per feats sub-tile (10)

_CACHE = {}


def _consts():
    import ml_dtypes
    bf16 = ml_dtypes.bfloat16
    f16 = np.float16
    q = np.arange(128)
    g = q // 16       # row-group of partition
    w = q % 16        # within-group lane (part index / wrap residue)

    c = {}
    c["E8H"] = (g[None, :] == np.arange(8)[:, None]).astype(f16)             # [8,128]
    c["G2H"] = (g[:, None] == g[None, :]).astype(f16)                        # [128,128]
    c["T16"] = ((g[:, None] == g[None, :]) & (w[:, None] < w[None, :])).astype(np.float32)
    c["R16"] = (w[:, None] == np.arange(16)[None, :]).astype(np.float32)     # [128,16]
    # EEr[r][q, q'] = (q == 16 r + q'%16): broadcast row-r's 16-part stripe to all 128
    for r in range(8):
        c[f"EErb{r}"] = (q[:, None] == 16 * r + w[None, :]).astype(bf16)
    c["iotaP"] = w.astype(np.float32)[:, None]                               # [128,1]
    c["iotaWn"] = (-w.astype(np.float32))[:, None]                           # [128,1]
    c["SIXT"] = np.full((128, 1), 0.0625, np.float32)
    c["JDATA"] = np.broadcast_to(np.arange(K, dtype=np.int16), (128, K)).copy()
    c["NEG16"] = np.full((128, 16), -1e30, bf16)
    c["ONE1"] = np.ones((128, 1), np.float32)
    c["NEG1"] = np.full((128, 1), -1.0, np.float32)
    # block-level helpers
    c["A3H"] = (w[:, None] == w[None, :]).astype(f16)                        # [128,128]
    c["GR8H"] = (g[:, None] == np.arange(8)[None, :]).astype(f16)            # [128,8]
    c["ONES128H"] = np.ones((128, 128), f16)
    c["I128H"] = np.eye(128, dtype=f16)
    # ends-gather column pattern: idx[q, r] = 512*r (+ endsT)
    c["PATC"] = np.broadcast_to(512.0 * np.arange(8, dtype=np.float32), (128, 8)).copy()
    # one-hot compare pattern [128, (chunk, row, part)] -> iota over part
    c["IOTA16R"] = np.broadcast_to(
        np.tile(np.arange(16, dtype=np.float32), 32), (128, 512)).copy()
    return c


F32_PACK = ["T16", "R16", "iotaP", "iotaWn", "SIXT", "ONE1", "NEG1", "PATC",
            "IOTA16R"]
F16_PACK = ["G2H", "A3H", "ONES128H", "I128H", "GR8H"]
BF_PACK = ["NEG16"] + [f"EErb{r}" for r in range(8)]


def build_kernel_body(stk, tc, nc, dram):
    from concourse import mybir
    from concourse.tile_rust import add_dep_helper
    dt = mybir.dt
    Alu = mybir.AluOpType
    Act = mybir.ActivationFunctionType
    f32, i16, i32, bf, f16 = (dt.float32, dt.int16, dt.int32, dt.bfloat16,
                              dt.float16)

    feats_d = dram["feats"]     # [N_PER_CORE, C, S, K] f32
    labels_d = dram["labels"]   # [ROWS, K] f16 (host pre-cast)
    labelsT_d = dram["labelsT"]  # [NBLK, 128, 32] f16 transposed labels
    out_d = dram["out"]         # [N_PER_CORE, C, S, P] f32

    cpool = stk.enter_context(tc.tile_pool(name="consts", bufs=1))
    keep = stk.enter_context(tc.tile_pool(name="keep", bufs=9))
    lp = stk.enter_context(tc.tile_pool(name="lp", bufs=2))
    # PSUM pools (8 banks of 2KB/partition total):
    ppb = stk.enter_context(tc.tile_pool(name="ppb", bufs=2, space="PSUM"))
    ppo = stk.enter_context(tc.tile_pool(name="ppo", bufs=1, space="PSUM"))
    ivpool = stk.enter_context(tc.tile_pool(name="ivp", bufs=1, space="PSUM"))
    brpool = stk.enter_context(tc.tile_pool(name="brp", bufs=1, space="PSUM"))
    xtpool = stk.enter_context(tc.tile_pool(name="xtp", bufs=2, space="PSUM"))
    smpool = stk.enter_context(tc.tile_pool(name="smp", bufs=1, space="PSUM"))
    fpool = stk.enter_context(tc.tile_pool(name="feats", bufs=3))
    hpool = stk.enter_context(tc.tile_pool(name="featsh", bufs=2))
    vp = stk.enter_context(tc.tile_pool(name="vp", bufs=4))
    xspool = stk.enter_context(tc.tile_pool(name="xsb", bufs=2))
    scpool = stk.enter_context(tc.tile_pool(name="scp", bufs=2))
    opool = stk.enter_context(tc.tile_pool(name="outacc", bufs=2))

    cn = _CACHE["consts"]
    cmap = {}
    for blob, names, dtp in (("CF32A", F32A_PACK, f32), ("CF16", F16_PACK, f16),
                             ("CF32B", F32B_PACK, f32), ("CBF", BF_PACK, bf)):
        w = sum(cn[n].shape[1] for n in names)
        t = cpool.tile([128, w], dtp, tag=blob)
        nc.sync.dma_start(out=t[:], in_=dram[blob][:])
        o = 0
        for n in names:
            wn = cn[n].shape[1]
            cmap[n] = t[:, o:o + wn]
            o += wn
    JD = cpool.tile([128, K], i16, tag="JD")
    nc.sync.dma_start(out=JD[:], in_=dram["JDATA"][:])
    E8t = cpool.tile([8, 128], f16, tag="E8t")
    nc.sync.dma_start(out=E8t[:], in_=dram["E8H"][:])

    class _V:
        def __init__(self, ap):
            self.ap = ap
        def __getitem__(self, sl):
            if sl == slice(None):
                return self.ap
            return self.ap[sl]

    E8H = _V(E8t[:])
    JDATA = _V(JD[:])
    G2H = _V(cmap["G2H"])
    T16 = _V(cmap["T16"])
    R16 = _V(cmap["R16"])
    iotaP = _V(cmap["iotaP"])
    iotaWn = _V(cmap["iotaWn"])
    SIXT = _V(cmap["SIXT"])
    NEG16 = _V(cmap["NEG16"])
    ONE1 = _V(cmap["ONE1"])
    NEG1 = _V(cmap["NEG1"])
    A3H = _V(cmap["A3H"])
    GR8H = _V(cmap["GR8H"])
    ONES128H = _V(cmap["ONES128H"])
    I128H = _V(cmap["I128H"])
    PATC = _V(cmap["PATC"])
    IOTA16R = _V(cmap["IOTA16R"])
    EErb = [_V(cmap[f"EErb{r}"]) for r in range(8)]

    KDEBUG = bool(os.environ.get("KDEBUG"))
    def dbg_dump(name, tile_ap):
        if KDEBUG and name in dram:
            nc.sync.dma_start(out=dram[name][:], in_=tile_ap)

    # ---------------- phase 1: label pipeline per block ----------------
    blocks = {}
    scatter_insts = []
    last_gather = [None]

    def label_block(b):
        Lf8 = lp.tile([BLK, K], f16, tag="Lf8")
        nc.sync.dma_start(out=Lf8[:], in_=labels_d[b * BLK:(b + 1) * BLK, :])
        Lrep = ppb.tile([128, K], f32, tag="big")
        nc.tensor.matmul(Lrep[:], lhsT=E8H[:], rhs=Lf8[:], start=True, stop=True)

        # one-hot: O = (Lrep == p(w))  -- in1 unused under bypass
        O = lp.tile([128, K], f16, tag="O")
        nc.vector.scalar_tensor_tensor(
            out=O[:], in0=Lrep[:], scalar=iotaP[:, 0:1],
            in1=iotaP[:, 0:1].to_broadcast([128, K]),
            op0=Alu.is_equal, op1=Alu.bypass)

        # cumulative count along k
        Cc = lp.tile([128, K], f16, tag="Cc")
        nc.vector.tensor_tensor_scan(
            out=Cc[:], data0=O[:], data1=O[:], initial=0.0,
            op0=Alu.add, op1=Alu.bypass)
        counts = Cc[:, K - 1:K]

        # mrgall packs small PSUM outputs:
        # [0:8) endsT, [8:9) offsets, [16:32) offT, [32:288) ribc diag
        mrgall = ppo.tile([128, 288], f32, tag="mrgall")
        countsf = lp.tile([128, 1], f32, tag="countsf")
        nc.vector.tensor_copy(out=countsf[:], in_=counts)
        offps = mrgall[:, 8:9]
        nc.tensor.matmul(offps[:], lhsT=T16[:], rhs=countsf[:], start=True, stop=True)

        om1 = lp.tile([128, 1], f32, tag="om1")
        nc.vector.tensor_scalar(out=om1[:], in0=offps[:], scalar1=-1.0,
                                scalar2=None, op0=Alu.add)
        ends0 = lp.tile([128, 1], f32, tag="ends0")
        nc.vector.tensor_tensor(out=ends0[:], in0=om1[:], in1=countsf[:], op=Alu.add)
        endsc = lp.tile([128, 1], f32, tag="endsc")
        nc.vector.tensor_scalar(out=endsc[:], in0=ends0[:], scalar1=0.0,
                                scalar2=None, op0=Alu.max)

        ctc = lp.tile([128, 1], f32, tag="ctc")
        nc.vector.tensor_scalar(out=ctc[:], in0=countsf[:], scalar1=1.0,
                                scalar2=None, op0=Alu.max)
        recip = lp.tile([128, 1], f16, tag="recip")
        with nc.allow_low_precision(reason="recip of small-int counts; f16 rel err 5e-4 ok"):
            nc.vector.reciprocal(out=recip[:], in_=ctc[:])
        indic = lp.tile([128, 1], f16, tag="indic")
        nc.vector.tensor_scalar(out=indic[:], in0=countsf[:], scalar1=0.0,
                                scalar2=None, op0=Alu.is_gt)

        # diag forms: one ONES128H matmul broadcasts recip/indic over (r,p)
        ridiag = lp.tile([128, 256], f16, tag="ridiag")
        nc.vector.tensor_tensor(out=ridiag[:, 0:128],
                                in0=recip[:, 0:1].to_broadcast([128, 128]),
                                in1=I128H[:], op=Alu.mult)
        nc.vector.tensor_tensor(out=ridiag[:, 128:256],
                                in0=indic[:, 0:1].to_broadcast([128, 128]),
                                in1=I128H[:], op=Alu.mult)
        ribc = keep.tile([128, 256], f16, tag="ribc")
        mrg = mrgall[:, 32:288]
        nc.tensor.matmul(mrg[:], lhsT=ONES128H[:], rhs=ridiag[:], start=True, stop=True)
        nc.scalar.copy(out=ribc[:], in_=mrg[:])

        # ends transposed to [w-partition, r-free], then gather idx = 512r + end
        e8d = lp.tile([128, 8], f16, tag="e8d")
        nc.vector.tensor_tensor(out=e8d[:], in0=endsc[:, 0:1].to_broadcast([128, 8]),
                                in1=GR8H[:], op=Alu.mult)
        endsT = mrgall[:, 0:8]
        nc.tensor.matmul(endsT[:], lhsT=A3H[:], rhs=e8d[:], start=True, stop=True)
        eidxf = lp.tile([128, 8], f32, tag="eidxf")
        nc.vector.tensor_tensor(out=eidxf[:], in0=endsT[:], in1=PATC[:], op=Alu.add)
        eidx = keep.tile([128, 8], i16, tag="eidx")
        nc.scalar.activation(out=eidx[:], in_=eidxf[:], func=Act.Copy)

        # transposed labels for the sum one-hot (host-prepared layout)
        labTs = lp.tile([128, 32], f16, tag="labTs")
        nc.sync.dma_start(out=labTs[:], in_=labelsT_d[b])
        # OHb[kp, (chunk, row, part)] = (labT[kp, chunk, row] == part)
        OHb = keep.tile([128, 512], f16, tag="OHb")
        nc.vector.tensor_tensor(
            out=OHb[:],
            in0=labTs[:].rearrange("q (f one) -> q f one", one=1).to_broadcast([128, 32, 16]),
            in1=IOTA16R[:].rearrange("q (f p) -> q f p", p=16),
            op=Alu.is_equal)

        # positions: posm = (Cc + (off-1)) * O   (masked; zero elsewhere)
        posm = lp.tile([128, K], f16, tag="posm")
        nc.vector.scalar_tensor_tensor(
            out=posm[:], in0=Cc[:], scalar=om1[:, 0:1], in1=O[:],
            op0=Alu.add, op1=Alu.mult)
        posr = ppb.tile([128, K], f32, tag="big")
        nc.tensor.matmul(posr[:], lhsT=G2H[:], rhs=posm[:], start=True, stop=True)

        # wrapped-inverse index build (rounding-mode independent):
        # e = (pos - w)/16 is integer iff partition lane w owns sorted slot pos
        ev = lp.tile([128, K], f16, tag="ev")
        nc.vector.scalar_tensor_tensor(
            out=ev[:], in0=posr[:], scalar=iotaWn[:, 0:1],
            in1=SIXT[:, 0:1].to_broadcast([128, K]),
            op0=Alu.add, op1=Alu.mult)
        ei = lp.tile([128, K], i16, tag="ei")
        nc.scalar.activation(out=ei[:], in_=ev[:], func=Act.Copy)
        efp1 = lp.tile([128, K], f16, tag="efp1")
        nc.scalar.activation(out=efp1[:], in_=ei[:], func=Act.Identity, bias=ONE1[:, 0:1])
        # match = (round(ev) == ev) == (efp1 - 1 == ev), fused with the mult
        match = lp.tile([128, K], f16, tag="match")
        nc.vector.scalar_tensor_tensor(
            out=match[:], in0=efp1[:], scalar=-1.0, in1=ev[:],
            op0=Alu.add, op1=Alu.is_equal)
        idxwf = lp.tile([128, K], f16, tag="idxwf")
        nc.vector.tensor_tensor(out=idxwf[:], in0=match[:], in1=efp1[:], op=Alu.mult)
        idx16 = lp.tile([128, K], i16, tag="idx16")
        nc.scalar.activation(out=idx16[:], in_=idxwf[:], func=Act.Identity, bias=NEG1[:, 0:1])

        inv = lp.tile([128, K // 16], i16, tag="inv")
        sc_i1 = nc.gpsimd.local_scatter(
            out_ap=inv[:], data_ap=JDATA[:], idxs_ap=idx16[:],
            channels=128, num_elems=K // 16, num_idxs=K)

        # boundary stripe from offsets: offT = G2 @ diag-ish offsets
        offdh = lp.tile([128, 16], f16, tag="offdh")
        nc.vector.tensor_tensor(out=offdh[:], in0=offps[:, 0:1].to_broadcast([128, 16]),
                                in1=R16[:], op=Alu.mult)
        nc.tensor.matmul(mrgall[:, 16:32], lhsT=G2H[:], rhs=offdh[:],
                         start=True, stop=True)
        offT16 = lp.tile([128, 16], i16, tag="offT16")
        nc.scalar.activation(out=offT16[:], in_=mrgall[:, 16:32], func=Act.Copy)

        bneg = keep.tile([128, K], bf, tag="bneg")
        sc_i2 = nc.gpsimd.local_scatter(
            out_ap=bneg[:], data_ap=NEG16[:], idxs_ap=offT16[:],
            channels=128, num_elems=K, num_idxs=16)

        invf = lp.tile([128, K // 16], f16, tag="invf")
        nc.scalar.activation(out=invf[:], in_=inv[:], func=Act.Copy)
        # rhs8[q, (r,s)] = invf[q,s] * (group(q)==r); A3 matmul then yields
        # invall[q', (r,s)] = invf[16r + w(q'), s] = row-r's wrapped inverse
        # replicated to every core group.
        rhs8 = lp.tile([128, BLK * (K // 16)], f16, tag="rhs8")
        for rr_ in range(BLK):
            nc.vector.tensor_tensor(
                out=rhs8[:, rr_ * (K // 16):(rr_ + 1) * (K // 16)],
                in0=invf[:], in1=GR8H[:, rr_:rr_ + 1].to_broadcast([128, K // 16]),
                op=Alu.mult)
        invall_ps = ivpool.tile([128, BLK * (K // 16)], f32, tag="invall_ps")
        nc.tensor.matmul(invall_ps[:], lhsT=A3H[:], rhs=rhs8[:], start=True, stop=True)
        invall16 = keep.tile([128, BLK * (K // 16)], i16, tag="invall16")
        nc.vector.tensor_copy(out=invall16[:], in_=invall_ps[:])
        if b == 0:
            dbg_dump("d_O", O[:])
            dbg_dump("d_Cc", Cc[:])
            dbg_dump("d_posm", posm[:])
            dbg_dump("d_ev", ev[:])
            dbg_dump("d_idx16", idx16[:])
            dbg_dump("d_inv", inv[:])
            dbg_dump("d_offT16", offT16[:])
            dbg_dump("d_bneg", bneg[:])
            dbg_dump("d_ribc", ribc[:])
            dbg_dump("d_eidxB", eidx[:])
            dbg_dump("d_OHb", OHb[:])
        for sc in (sc_i1, sc_i2):
            if epoch_last_gather[0] is not None:
                add_dep_helper(sc.ins, epoch_last_gather[0].ins, False,
                               "pool library epoch order")
        scatter_insts.extend([sc_i1, sc_i2])
        blocks[b] = dict(invall16=invall16, eidx=eidx, ribc=ribc,
                         bneg=bneg, OHb=OHb)

    # ---------------- phase 2: value pipeline per row ----------------
    feats_tiles = {}
    featsh_tiles = {}
    out_tiles = {}

    def fetch_half(half):
        if half in feats_tiles:
            return
        ni_, h_ = half
        s0 = h_ * SH
        ft = fpool.tile([128, SH * K], f32, tag="ft")
        nc.sync.dma_start(
            out=ft[:],
            in_=feats_d[ni_, :, s0:s0 + SH, :].rearrange("c s k -> c (s k)"))
        feats_tiles[half] = ft
        fh = hpool.tile([128, SH * K], f16, tag="fh")
        nc.scalar.activation(out=fh[:], in_=ft[:], func=Act.Copy)
        featsh_tiles[half] = fh
    scano_blk = [None]
    xtp_cur = [None]
    xsb_cur = [None]
    smp_cur = [None]
    pending = {}

    def value_row(g_row):
        ni, si = g_row // S, g_row % S
        b, r = g_row // BLK, g_row % BLK
        bk = blocks[b]

        half = (ni, si // SH)
        fetch_half(half)
        ft = feats_tiles[half]
        fh = featsh_tiles[half]
        fs = (si % SH) * K

        invr16 = bk["invall16"][:, r * (K // 16):(r + 1) * (K // 16)]
        gath = vp.tile([128, K], f32, tag="gath")
        g_i = nc.gpsimd.ap_gather(
            out_ap=gath[:], in_ap=ft[:, fs:fs + K], idxs_ap=invr16,
            channels=128, num_elems=K, d=1, num_idxs=K)
        if scatter_insts:
            add_dep_helper(g_i.ins, scatter_insts[-1].ins, False,
                           "pool library phase order")

        # max-scan boundary row (PSUM f32 via bf16 broadcast matmul)
        brow = brpool.tile([128, K], f32, tag="brow")
        nc.tensor.matmul(brow[:], lhsT=EErb[r][:], rhs=bk["bneg"][:],
                         start=True, stop=True)

        if r == 0:
            sc_new = scpool.tile([128, BLK * K], f32, tag="scano")
            scano_blk[0] = sc_new
            sump_new = smpool.tile([128, 128], f32, tag="sump")
            smp_cur[0] = sump_new
        scano = scano_blk[0]
        sump = smp_cur[0]
        nc.vector.tensor_tensor_scan(
            out=scano[:, r * K:(r + 1) * K], data0=brow[:], data1=gath[:],
            initial=0.0, op0=Alu.add, op1=Alu.max)

        # ---- sum path: transpose fp16 chunks -> PSUM, evac pairs, matmul ----
        if r % 2 == 0:
            xtp_new = xtpool.tile([128, 1024], f16, tag="xtp")
            xtp_cur[0] = xtp_new
        xtp = xtp_cur[0]
        xo = (r % 2) * K
        for ch in range(4):
            nc.tensor.matmul(xtp[:, xo + ch * 128:xo + (ch + 1) * 128],
                             lhsT=fh[:, fs + ch * 128:fs + (ch + 1) * 128],
                             rhs=I128H[:], is_transpose=True,
                             start=True, stop=True)
        if r % 2 == 1:
            xsb = xspool.tile([128, 1024], f16, tag="xsb")
            nc.scalar.copy(out=xsb[:], in_=xtp[:])
            xsb_cur[0] = xsb
            for rr in (r - 1, r):
                xs = ((rr % 2)) * K
                for ch in range(4):
                    nc.tensor.matmul(
                        sump[:, rr * 16:(rr + 1) * 16],
                        lhsT=xsb[:, xs + ch * 128:xs + (ch + 1) * 128],
                        rhs=bk["OHb"][:, ch * 128 + rr * 16:ch * 128 + (rr + 1) * 16],
                        start=(ch == 0), stop=(ch == 3))

        if g_row == 0:
            dbg_dump("d_gath", gath[:])
            dbg_dump("d_scano", scano[:, 0:K])
        if r == BLK - 1:
            pending[b] = (scano, sump)

    def finish_block(b):
        scano, sump = pending.pop(b)
        bk = blocks[b]
        # gather all 8 rows' max segment-end values: [c, (r, p)]
        gath2 = vp.tile([128, 128], f32, tag="gath2")
        if b == NBLK - 1:
            # last block: split into per-pair quarters so each piece only
            # waits for its own rows' scans instead of scan r7
            eidx_rb = lp.tile([128, 8], i16, tag="eidx_rb")
            for h in range(4):
                nc.vector.tensor_scalar(
                    out=eidx_rb[:, 2 * h:2 * h + 2],
                    in0=bk["eidx"][:, 2 * h:2 * h + 2],
                    scalar1=float(-2 * K * h), scalar2=None, op0=Alu.add)
            for h in range(4):
                g2_i = nc.gpsimd.ap_gather(
                    out_ap=gath2[:, 32 * h:32 * h + 32],
                    in_ap=scano[:, 2 * K * h:2 * K * (h + 1)],
                    idxs_ap=eidx_rb[:, 2 * h:2 * h + 2],
                    channels=128, num_elems=2 * K, d=1, num_idxs=32)
        else:
            g2_i = nc.gpsimd.ap_gather(
                out_ap=gath2[:], in_ap=scano[:], idxs_ap=bk["eidx"][:],
                channels=128, num_elems=BLK * K, d=1, num_idxs=128)
        if scatter_insts:
            add_dep_helper(g2_i.ins, scatter_insts[-1].ins, False,
                           "pool op order")
        last_gather[0] = g2_i
        t1 = vp.tile([128, 128], f32, tag="t1")
        nc.vector.tensor_tensor(out=t1[:], in0=sump[:],
                                in1=bk["ribc"][:, 0:128], op=Alu.mult)
        t2 = vp.tile([128, 128], f32, tag="t2")
        nc.vector.tensor_tensor(out=t2[:], in0=gath2[:],
                                in1=bk["ribc"][:, 128:256], op=Alu.mult)
        if b == 0:
            dbg_dump("d_gath2", gath2[:])
            dbg_dump("d_t1", t1[:, 0:P])
            dbg_dump("d_t2", t2[:, 0:P])
        # write combined rows into out accumulators (split at n boundary)
        row0 = b * BLK
        r_off = 0
        while r_off < BLK:
            gr = row0 + r_off
            ni2, si2 = gr // S, gr % S
            span = min(BLK - r_off, S - si2)
            if ni2 not in out_tiles:
                ot_n = opool.tile([128, S * P], f32, tag="ot")
                out_tiles[ni2] = ot_n
            ot2 = out_tiles[ni2]
            nc.vector.tensor_tensor(
                out=ot2[:, si2 * P:(si2 + span) * P],
                in0=t1[:, r_off * P:(r_off + span) * P],
                in1=t2[:, r_off * P:(r_off + span) * P], op=Alu.add)
            if si2 + span == S:
                nc.sync.dma_start(out=out_d[ni2].rearrange("c s p -> c (s p)"),
                                  in_=ot2[:])
            r_off += span

    # ---------------- interleaved driver ----------------
    # Software-pipelined: label chains for block b+2 are emitted while block
    # b's rows stream through the value pipeline; each block's ends-gather is
    # deferred past the next block's first row so the Pool queue never stalls
    # on the last scan.
    label_compute(0)
    label_scatter(0)
    label_compute(1)
    for g_ in range(ROWS):
        b_, r_ = g_ // BLK, g_ % BLK
        value_row(g_)
        gpf = g_ + 5
        if gpf < ROWS:
            fetch_half((gpf // S, (gpf % S) // SH))
        if r_ == 2:
            if b_ >= 1:
                finish_block(b_ - 1)
        if r_ == 1 and b_ + 2 < NBLK:
            label_compute(b_ + 2)
        if r_ == 3 and b_ + 1 < NBLK:
            label_scatter(b_ + 1)
    finish_block(NBLK - 1)


def build_nc():
    if "nc" in _CACHE:
        return _CACHE["nc"]
    from concourse import bacc, mybir, tile
    dt = mybir.dt
    cn = _consts()
    _CACHE["consts"] = cn
    nc = bacc.Bacc("TRN2", target_bir_lowering=False, debug=False,
                   enable_asserts=False, num_devices=N_CORES)
    dram = {}
    dram["feats"] = nc.dram_tensor("feats", [N_PER_CORE, C, S, K], dt.float32,
                                   kind="ExternalInput").ap()
    dram["labels"] = nc.dram_tensor("labels", [ROWS, K], dt.float16,
                                    kind="ExternalInput").ap()
    dram["labelsT"] = nc.dram_tensor("labelsT", [NBLK, 128, 32], dt.float16,
                                     kind="ExternalInput").ap()
    dram["out"] = nc.dram_tensor("out", [N_PER_CORE, C, S, P], dt.float32,
                                 kind="ExternalOutput").ap()

    def dtf(a):
        if a.dtype == np.int16:
            return dt.int16
        if str(a.dtype) == "bfloat16":
            return dt.bfloat16
        if a.dtype == np.float16:
            return dt.float16
        return dt.float32

    packs = {"CF32A": (F32A_PACK, dt.float32), "CF32B": (F32B_PACK, dt.float32),
             "CF16": (F16_PACK, dt.float16), "CBF": (BF_PACK, dt.bfloat16)}
    for blob, (names, dtp) in packs.items():
        w = sum(cn[n].shape[1] for n in names)
        dram[blob] = nc.dram_tensor(f"c_{blob}", [128, w], dtp,
                                    kind="ExternalInput").ap()
    dram["JDATA"] = nc.dram_tensor("c_JDATA", [128, K], dt.int16,
                                   kind="ExternalInput").ap()
    dram["E8H"] = nc.dram_tensor("c_E8H", [8, 128], dt.float16,
                                 kind="ExternalInput").ap()

    if os.environ.get("KDEBUG"):
        dbg_specs = {
            "d_O": ([128, K], dt.float16), "d_Cc": ([128, K], dt.float16),
            "d_posm": ([128, K], dt.float16), "d_ev": ([128, K], dt.float16),
            "d_idx16": ([128, K], dt.int16), "d_inv": ([128, K // 16], dt.int16),
            "d_offT16": ([128, 16], dt.int16), "d_bneg": ([128, K], dt.bfloat16),
            "d_ribc": ([128, 256], dt.float16), "d_eidxB": ([128, 8], dt.int16),
            "d_OHb": ([128, 512], dt.float16),
            "d_gath": ([128, K], dt.float32),
            "d_scano": ([128, K], dt.float32),
            "d_gath2": ([128, 128], dt.float32),
            "d_t1": ([128, P], dt.float32), "d_t2": ([128, P], dt.float32),
        }
        for k, (shp, d) in dbg_specs.items():
            dram[k] = nc.dram_tensor(k, shp, d, kind="ExternalOutput").ap()

    with tile.TileContext(nc) as tc:
        with ExitStack() as stk:
            build_kernel_body(stk, tc, nc, dram)
    nc.compile()
    _CACHE["nc"] = nc
    import ml_dtypes
    _CACHE["blob32a"] = np.concatenate([cn[n].astype(np.float32) for n in F32A_PACK], axis=1)
    _CACHE["blob32b"] = np.concatenate([cn[n].astype(np.float32) for n in F32B_PACK], axis=1)
    _CACHE["blob16"] = np.concatenate([cn[n].astype(np.float16) for n in F16_PACK], axis=1)
    _CACHE["blobbf"] = np.concatenate([cn[n].astype(ml_dtypes.bfloat16) for n in BF_PACK], axis=1)
    return nc


def _host_fallback(feats, part_labels, valid_mask, parts_num):
    n, c, s, k = feats.shape
    Pn = int(parts_num)
    f = np.asarray(feats, np.float32).transpose(0, 2, 3, 1).reshape(-1, c)
    seg = (np.asarray(part_labels).astype(np.int64).reshape(n * s, k)
           + np.arange(n * s, dtype=np.int64)[:, None] * Pn).reshape(-1)
    vm = np.asarray(valid_mask).reshape(-1).astype(np.float32)
    nsg = n * s * Pn
    psum = np.zeros((nsg, c), np.float32)
    np.add.at(psum, seg, f * vm[:, None])
    pcnt = np.zeros(nsg, np.float32)
    np.add.at(pcnt, seg, vm)
    patch = np.zeros(nsg, np.float32)
    np.add.at(patch, seg, np.ones_like(vm))
    smax = np.full((nsg, c), -np.inf, np.float32)
    np.maximum.at(smax, seg, f)
    pmax = np.where(patch[:, None] > 0, np.maximum(smax, -100.0), 0.0)
    pooled = psum / np.maximum(pcnt, 1.0)[:, None] + pmax
    return pooled.reshape(n, s, Pn, c).transpose(0, 3, 1, 2).astype(np.float32)


def kernel(feats, part_labels, valid_mask, parts_num):
    feats = np.ascontiguousarray(np.asarray(feats), dtype=np.float32)
    if int(parts_num) != P or feats.shape != (N, C, S, K) \
            or not bool(np.all(np.asarray(valid_mask))):
        return _host_fallback(feats, part_labels, valid_mask, parts_num)

    import ml_dtypes
    from concourse import bass_utils
    nc = build_nc()
    cn = _CACHE["consts"]
    labels_bf = np.asarray(part_labels).astype(np.float16)

    in_maps = []
    for core in range(N_CORES):
        sl = slice(core * N_PER_CORE, (core + 1) * N_PER_CORE)
        lab_core = np.ascontiguousarray(labels_bf[sl]).reshape(ROWS, K)
        # labelsT[b, q, c*8 + r] = labels[b*8 + r, c*128 + q]
        labT = lab_core.reshape(NBLK, BLK, 4, 128).transpose(0, 3, 2, 1)
        m = {"feats": np.ascontiguousarray(feats[sl]),
             "labels": lab_core,
             "labelsT": np.ascontiguousarray(labT).reshape(NBLK, 128, 32)}
        m["c_CF32A"] = _CACHE["blob32a"]
        m["c_CF32B"] = _CACHE["blob32b"]
        m["c_CF16"] = _CACHE["blob16"]
        m["c_CBF"] = _CACHE["blobbf"]
        m["c_JDATA"] = cn["JDATA"]
        m["c_E8H"] = cn["E8H"]
        in_maps.append(m)

    res = bass_utils.run_bass_kernel_spmd(nc, in_maps, core_ids=list(range(N_CORES)))
    out = np.empty((N, C, S, P), np.float32)
    for core in range(N_CORES):
        out[core * N_PER_CORE:(core + 1) * N_PER_CORE] = res.results[core]["out"]
    return out
